# revision 29
# baseline (speedup 1.0000x reference)
"""MoE layer (16 experts, top-2) + shared SwiGLU MLP on 8 trn2 NeuronCores.

Sharding:
  - MoE experts: expert-parallel — core c owns experts {2c, 2c+1}. The host
    computes the router (0.2% of the FLOPs), gathers each expert's tokens
    (the "all-to-all" happens while building per-core inputs), and the device
    runs both expert FFNs on the gathered tokens.
  - Shared SwiGLU MLP: hybrid 4-way token x 2-way hidden shard. Core c
    handles token quarter (c % 4) and S-half (c // 4); each core emits a
    partial second-matmul output and the host sums the two S-halves.
  - The host applies the top-2 softmax combine weights, scatter-adds expert
    outputs, and adds the shared-expert output.

All matmul operands are bf16 (PSUM accumulates fp32): halves HBM traffic vs
fp32 and enables the PE's automatic fast-weight-load path, and the 2e-2
rel-err budget dwarfs the ~1e-3 bf16 error. Device outputs are bf16 too
(host accumulates in fp32). Every input tensor gets a dedicated SBUF tile
whose DMA is issued up front (sync engine: shared-expert stream; gpsimd:
expert weights + second-layer weights), so the tensor engine never waits on
a recycled buffer. Every operand is laid out host-side exactly as its SBUF
tile (partition-major), so each DMA is one contiguous-row transfer and every
matmul is lhsT.T @ rhs with no on-device transposes.
"""

import os
import numpy as np

import concourse.bacc as bacc
import concourse.mybir as mybir
import concourse.tile as tile
from concourse import bass_utils

AF = mybir.ActivationFunctionType
FP32 = mybir.dt.float32

B, L, D, H, E, S = 2, 2048, 1024, 512, 16, 2048
T = B * L
TOP_K = 2
NCORES = 8
EPC = E // NCORES   # experts per core
PT = 4              # token-shard ways for the shared expert
PS = 2              # hidden(S)-shard ways for the shared expert
TQ = T // PT        # tokens per core for the shared expert (1024)
SH = S // PS        # hidden units per core for the shared expert (1024)

KD = D // 128       # 8 contraction tiles over D
KH = H // 128       # 4 contraction tiles over H
KSH = SH // 128     # 8 s-tiles per core (its S-half)

MM_DTYPE = os.environ.get("KMM_DTYPE", "bf16")
_MM_DT = {
    "fp32": mybir.dt.float32,
    "fp32r": mybir.dt.float32r,
    "bf16": mybir.dt.bfloat16,
}

TRACE = False      # set True (or BASS_TRACE=1) to collect an NTFF profile
LAST = None        # BassKernelResults of the most recent run (for test.py)

_PROG_CACHE = {}


def _chunks(total, step=512):
    """Split ``total`` into near-equal chunks <= step (PSUM bank = 512 fp32)."""
    n = max(1, -(-total // step))
    base = total // n
    rem = total - base * n
    out, off = [], 0
    for i in range(n):
        w = base + (1 if i < rem else 0)
        out.append((off, w))
        off += w
    return out


def _pmajor(a, cols):
    """[K, M] k-major matrix -> [128, (K/128)*M] partition-major image whose
    columns are the K-tiles side by side; ``cols`` = M per tile."""
    K, M = a.shape
    assert M == cols
    return np.ascontiguousarray(
        a.reshape(K // 128, 128, M).transpose(1, 0, 2).reshape(128, -1)
    )


def build_program(C0, C1, mmdt_key=None):
    mmdt = _MM_DT[mmdt_key or MM_DTYPE]
    outdt = mmdt if mmdt == mybir.dt.bfloat16 else FP32
    nc = bacc.Bacc(
        "TRN2", target_bir_lowering=False, debug=False, enable_asserts=False
    )

    CS = (C0, C1)
    xgw = KD * (C0 + C1)

    # [128, (tc*KD + j)*512 + t]: token chunk tc, k-tile j, partition-major
    xq = nc.dram_tensor("xq", [128, 2 * KD * 512], mmdt, kind="ExternalInput").ap()
    # per s-tile: 8 sfc1 k-tiles then 8 sfc2 k-tiles, side by side
    sfc12 = nc.dram_tensor("sfc12", [KSH, 128, 2 * KD * 128], mmdt, kind="ExternalInput").ap()
    # [128, dt*KSH*128 + s]: the core's 8 sfc3 s-tiles per d-tile
    sfc3h = nc.dram_tensor("sfc3h", [128, KD * KSH * 128], mmdt, kind="ExternalInput").ap()
    # slot-0 (wide) expert block [128, KD*C0], then slot-1 block [128, KD*C1]
    xg = nc.dram_tensor("xg", [128, xgw], mmdt, kind="ExternalInput").ap()
    w1b = nc.dram_tensor("w1b", [EPC, 128, KH * KD * 128], mmdt, kind="ExternalInput").ap()
    w2b = nc.dram_tensor("w2b", [EPC, 128, KD * KH * 128], mmdt, kind="ExternalInput").ap()
    pshout = nc.dram_tensor("pshout", [KD, 128, TQ], outdt, kind="ExternalOutput").ap()
    yout = nc.dram_tensor("yout", [128, xgw], outdt, kind="ExternalOutput").ap()

    tch = _chunks(TQ)   # token chunks for the shared expert (2 x 512)
    cchs = (_chunks(C0), _chunks(C1))   # token chunks for the owned experts

    with tile.TileContext(nc) as tc:
        with (
            tc.tile_pool(name="inp", bufs=1) as inp,
            tc.tile_pool(name="gp", bufs=1) as gp,
            tc.tile_pool(name="hp", bufs=1) as hp,
            tc.tile_pool(name="sap", bufs=3) as sap,
            tc.tile_pool(name="obp", bufs=1) as obp,
            tc.tile_pool(name="pop", bufs=3) as pop,
            tc.tile_pool(name="ps", bufs=8, space="PSUM") as ps,
        ):
            # ---- all input DMAs up front into dedicated tiles, one ordered
            # stream on the sync engine (issue order = rough arrival order,
            # which matches first-use order; a second engine's stream would
            # race this one for HBM bandwidth and starve stage A) ----
            s12t = [None] * KSH
            for st in range(KSH):
                s12t[st] = inp.tile([128, 2 * KD * 128], mmdt, tag=f"s12_{st}", name=f"s12_{st}")

            xq_t = inp.tile([128, 2 * KD * 512], mmdt, tag="xq", name="xq_t")

            def xq_sl(j, off, w):
                tc_i, local = off // 512, off % 512
                base = (tc_i * KD + j) * 512 + local
                return xq_t[:, base:base + w]

            xe_t, w1t, w2t = [None, None], [None, None], [None, None]
            for e in range(EPC):
                xe_t[e] = inp.tile([128, KD * CS[e]], mmdt, tag=f"xe{e}", name=f"xe{e}")
                w1t[e] = inp.tile([128, KH * KD * 128], mmdt, tag=f"w1_{e}", name=f"w1_{e}")
                w2t[e] = inp.tile([128, KD * KH * 128], mmdt, tag=f"w2_{e}", name=f"w2_{e}")
            w3t = inp.tile([128, KD * KSH * 128], mmdt, tag="w3", name="w3t")

            # single ordered stream on sync (a second engine's stream races
            # this one for HBM bandwidth). Order = just-in-time arrival for
            # a warm PE that starts at the first group's data gate (~13.5us).
            # Note: do NOT split these transfers finer — partially-written
            # tiles make the scheduler add per-matmul subtile waits that
            # cost ~40ns on every downstream matmul (+25us total).
            qcol = KD * 512
            nc.sync.dma_start(out=s12t[0][:], in_=sfc12[0])
            nc.sync.dma_start(out=xq_t[:, :qcol], in_=xq[:, :qcol])
            nc.sync.dma_start(out=xq_t[:, qcol:], in_=xq[:, qcol:])
            for st in range(1, KSH):
                nc.sync.dma_start(out=s12t[st][:], in_=sfc12[st])
            nc.sync.dma_start(out=xe_t[0][:], in_=xg[:, : KD * C0])
            nc.sync.dma_start(out=w1t[0][:], in_=w1b[0])
            nc.sync.dma_start(out=w2t[0][:], in_=w2b[0])
            nc.sync.dma_start(out=w3t[:], in_=sfc3h[:])
            nc.sync.dma_start(out=xe_t[1][:], in_=xg[:, KD * C0:])
            nc.sync.dma_start(out=w1t[1][:], in_=w1b[1])
            nc.sync.dma_start(out=w2t[1][:], in_=w2b[1])

            # warm-up: ~6us of dependency-free matmuls (on gpsimd-memset
            # tiles) bridge the PE through the first DMA wait so the HAM
            # clock gate opens (1.2 -> 2.4 GHz) right as real work arrives;
            # sized to end at the first group's data gate, because a >2us
            # PE idle gap here would re-throttle the clock.
            dmw = inp.tile([128, 128], mmdt, tag="dmw", name="dmw")
            dmx = inp.tile([128, 512], mmdt, tag="dmx", name="dmx")
            nc.gpsimd.memset(dmw[:], 0.0)
            nc.gpsimd.memset(dmx[:], 0.0)
            # 19 iterations: ~8 run cold (427ns) before the HAM opens
            # mid-burst and the rest run warm (213ns), ending at ~12.3-14.2us
            # across HAM-phase jitter — just under the 13.6-14.5us data gate,
            # so the PE enters the real stream warm with no idle window.
            for _ in range(19):
                pd = ps.tile([128, 512], FP32, tag="ps")
                nc.tensor.matmul(pd[:], dmw[:], dmx[:], start=True, stop=True)

            # ---- stage A: g[s, t] = silu(x@sfc1.T) * (x@sfc2.T) ----
            g_t = gp.tile([128, KSH * TQ], mmdt, tag="g", name="g_t")
            for st in range(KSH):
                for off, w in tch:
                    pa = ps.tile([128, 512], FP32, tag="ps")
                    for j in range(KD):
                        nc.tensor.matmul(
                            pa[:, :w], s12t[st][:, j * 128:(j + 1) * 128],
                            xq_sl(j, off, w),
                            start=(j == 0), stop=(j == KD - 1),
                        )
                    sa = sap.tile([128, 512], FP32, tag="sa")
                    nc.scalar.activation(sa[:, :w], pa[:, :w], AF.Silu)
                    pb = ps.tile([128, 512], FP32, tag="ps")
                    for j in range(KD):
                        nc.tensor.matmul(
                            pb[:, :w], s12t[st][:, (KD + j) * 128:(KD + j + 1) * 128],
                            xq_sl(j, off, w),
                            start=(j == 0), stop=(j == KD - 1),
                        )
                    nc.vector.tensor_mul(
                        g_t[:, st * TQ + off:st * TQ + off + w],
                        sa[:, :w], pb[:, :w],
                    )

            # ---- owned experts: y_e = silu(x_e @ w1.T) @ w2.T ----
            def emit_expert(e):
                C = CS[e]
                ybase = 0 if e == 0 else KD * C0
                h_t = hp.tile([128, KH * C], mmdt, tag=f"h{e}", name=f"h{e}")
                for ht in range(KH):
                    for off, w in cchs[e]:
                        ph = ps.tile([128, 512], FP32, tag="ps")
                        for j in range(KD):
                            nc.tensor.matmul(
                                ph[:, :w],
                                w1t[e][:, (ht * KD + j) * 128:(ht * KD + j + 1) * 128],
                                xe_t[e][:, j * C + off:j * C + off + w],
                                start=(j == 0), stop=(j == KD - 1),
                            )
                        nc.scalar.activation(
                            h_t[:, ht * C + off:ht * C + off + w], ph[:, :w], AF.Silu
                        )
                yo = obp.tile([128, KD * C], outdt, tag=f"yo{e}", name=f"yo{e}")
                for dt in range(KD):
                    for off, w in cchs[e]:
                        py = ps.tile([128, 512], FP32, tag="ps")
                        for j in range(KH):
                            nc.tensor.matmul(
                                py[:, :w],
                                w2t[e][:, (dt * KH + j) * 128:(dt * KH + j + 1) * 128],
                                h_t[:, j * C + off:j * C + off + w],
                                start=(j == 0), stop=(j == KH - 1),
                            )
                        nc.vector.tensor_copy(
                            yo[:, dt * C + off:dt * C + off + w], py[:, :w]
                        )
                        if e == 1 and dt == KD - 1:
                            # very last output: per-chunk DMAs so the exit
                            # barrier waits on a half-size final transfer
                            nc.scalar.dma_start(
                                out=yout[:, ybase + dt * C + off:ybase + dt * C + off + w],
                                in_=yo[:, dt * C + off:dt * C + off + w],
                            )
                    if not (e == 1 and dt == KD - 1):
                        nc.scalar.dma_start(
                            out=yout[:, ybase + dt * C:ybase + (dt + 1) * C],
                            in_=yo[:, dt * C:(dt + 1) * C],
                        )

            def emit_stage_b():
                # partial shared second matmul over this core's S-half:
                # pshout[d, t] = sum_{s in half} sfc3[d, s] * g[s, t]
                for dt in range(KD):
                    po = pop.tile([128, TQ], outdt, tag="po")
                    for off, w in tch:
                        pc = ps.tile([128, 512], FP32, tag="ps")
                        for sj in range(KSH):
                            nc.tensor.matmul(
                                pc[:, :w],
                                w3t[:, (dt * KSH + sj) * 128:(dt * KSH + sj + 1) * 128],
                                g_t[:, sj * TQ + off:sj * TQ + off + w],
                                start=(sj == 0), stop=(sj == KSH - 1),
                            )
                        nc.vector.tensor_copy(po[:, off:off + w], pc[:, :w])
                    nc.scalar.dma_start(out=pshout[dt], in_=po[:])

            emit_expert(0)
            emit_stage_b()
            emit_expert(1)

    nc.compile()
    return nc


def kernel(**inputs):
    global LAST
    x = np.ascontiguousarray(np.asarray(inputs["x"], dtype=np.float32))
    gate_w = np.asarray(inputs["gate_w"], dtype=np.float32)
    w1 = np.asarray(inputs["w1"], dtype=np.float32)
    w2 = np.asarray(inputs["w2"], dtype=np.float32)
    sfc1 = np.asarray(inputs["sfc1"], dtype=np.float32)
    sfc2 = np.asarray(inputs["sfc2"], dtype=np.float32)
    sfc3 = np.asarray(inputs["sfc3"], dtype=np.float32)

    xf = x.reshape(T, D)

    # router on host (tiny): top-2 of 16 logits, softmax over the pair
    logits = xf @ gate_w.T
    idx = np.argpartition(-logits, TOP_K, axis=1)[:, :TOP_K]
    lg = np.take_along_axis(logits, idx, axis=1)
    m = lg.max(axis=1, keepdims=True)
    p = np.exp(lg - m)
    wk = (p / p.sum(axis=1, keepdims=True)).astype(np.float32)

    toks, wts = [], []
    for e in range(E):
        sel = idx == e
        rows = np.nonzero(sel.any(axis=1))[0]
        toks.append(rows)
        wts.append(wk[sel])

    # slot assignment: the 8 most-loaded experts take slot 0 (width C0),
    # the 8 least-loaded take slot 1 (width C1 <= C0); core c gets
    # (order[c], order[E-1-c]). Shrinks the padded token count per core
    # from 2*max_load to max_load + median_load.
    order = sorted(range(E), key=lambda e: -len(toks[e]))
    assign = [(order[c], order[E - 1 - c]) for c in range(NCORES)]
    pad = lambda n: max(2 * ((n + 1) // 2), 256)
    C0 = pad(max(len(toks[e]) for e in order[:NCORES]))
    C1 = pad(max(len(toks[e]) for e in order[NCORES:]))
    CS = (C0, C1)

    key = (C0, C1, MM_DTYPE)
    if key not in _PROG_CACHE:
        _PROG_CACHE[key] = build_program(C0, C1, MM_DTYPE)
    nc = _PROG_CACHE[key]
    np_mm = mybir.dt.np(_MM_DT[MM_DTYPE])

    sfc1T = np.ascontiguousarray(sfc1.T)   # [D, S]
    sfc2T = np.ascontiguousarray(sfc2.T)
    sfc3T = np.ascontiguousarray(sfc3.T)   # [S, D]

    # sfc12 per S-half: [KSH, 128, 2*KD*128]
    sfc12_h, sfc3_h = [], []
    for sh in range(PS):
        blk = np.empty((KSH, 128, 2 * KD * 128), np.float32)
        for st in range(KSH):
            s0 = (sh * KSH + st) * 128
            a = sfc1T[:, s0:s0 + 128]    # [D, 128]
            b = sfc2T[:, s0:s0 + 128]
            blk[st, :, : KD * 128] = _pmajor(a, 128)
            blk[st, :, KD * 128:] = _pmajor(b, 128)
        sfc12_h.append(blk.astype(np_mm))
        blk3 = np.empty((KD, 128, KSH * 128), np.float32)
        s0 = sh * SH
        for dt in range(KD):
            # [SH, 128] slice of sfc3T -> partition-major over its s-tiles
            blk3[dt] = _pmajor(
                np.ascontiguousarray(sfc3T[s0:s0 + SH, dt * 128:(dt + 1) * 128]), 128
            )
        # [KD, 128, KSH*128] -> [128, KD*KSH*128] (dt-blocks side by side)
        sfc3_h.append(
            np.ascontiguousarray(
                blk3.transpose(1, 0, 2).reshape(128, KD * KSH * 128)
            ).astype(np_mm)
        )

    in_maps = []
    for c in range(NCORES):
        q, sh = c % PT, c // PT
        xqm = _pmajor(
            np.ascontiguousarray(xf[q * TQ:(q + 1) * TQ].T), TQ
        ).astype(np_mm)
        # [128, KD*TQ] -> [128, (tc*KD + j)*512 + t] (tc-major chunk blocks)
        xqm = np.ascontiguousarray(
            xqm.reshape(128, KD, 2, 512).transpose(0, 2, 1, 3).reshape(128, 2 * KD * 512)
        )
        xg_c = np.zeros((128, KD * (C0 + C1)), np.float32)
        w1_c, w2_c = [], []
        for k in range(EPC):
            e = assign[c][k]
            C = CS[k]
            base = 0 if k == 0 else KD * C0
            rows = toks[e]
            xe = np.zeros((C, D), np.float32)
            xe[: len(rows)] = xf[rows]
            xg_c[:, base:base + KD * C] = _pmajor(np.ascontiguousarray(xe.T), C)
            # w1 tiles keyed (ht, j): col block (ht*KD + j) is k-tile j of
            # w1[e].T's h-tile ht
            w1T = np.ascontiguousarray(w1[e].T)   # [D, H]
            w1m = np.empty((128, KH * KD * 128), np.float32)
            for ht in range(KH):
                w1m[:, ht * KD * 128:(ht + 1) * KD * 128] = _pmajor(
                    np.ascontiguousarray(w1T[:, ht * 128:(ht + 1) * 128]), 128
                )
            w1_c.append(w1m)
            # w2 tiles keyed (dt, hj)
            w2T = np.ascontiguousarray(w2[e].T)   # [H, D]
            w2m = np.empty((128, KD * KH * 128), np.float32)
            for dt in range(KD):
                w2m[:, dt * KH * 128:(dt + 1) * KH * 128] = _pmajor(
                    np.ascontiguousarray(w2T[:, dt * 128:(dt + 1) * 128]), 128
                )
            w2_c.append(w2m)
        in_maps.append(
            {
                "xq": xqm,
                "sfc12": sfc12_h[sh],
                "sfc3h": sfc3_h[sh],
                "xg": xg_c.astype(np_mm),
                "w1b": np.stack(w1_c).astype(np_mm),
                "w2b": np.stack(w2_c).astype(np_mm),
            }
        )

    trace = TRACE or os.environ.get("BASS_TRACE") == "1"
    res = bass_utils.run_bass_kernel_spmd(
        nc, in_maps, core_ids=list(range(NCORES)), trace=trace
    )
    LAST = res
    results = res.results

    out = np.empty((T, D), np.float32)
    for q in range(PT):
        acc = np.asarray(results[q]["pshout"], np.float32).reshape(D, TQ)
        acc = acc + np.asarray(results[PT + q]["pshout"], np.float32).reshape(D, TQ)
        out[q * TQ:(q + 1) * TQ] = acc.T
    for c in range(NCORES):
        yc = np.asarray(results[c]["yout"], np.float32)
        for k in range(EPC):
            e = assign[c][k]
            C = CS[k]
            base = 0 if k == 0 else KD * C0
            load = len(toks[e])
            # [128, KD*C] partition-major -> [D, C]
            yT = yc[:, base:base + KD * C].reshape(128, KD, C).transpose(1, 0, 2).reshape(D, C)
            out[toks[e]] += wts[e][:, None] * yT[:, :load].T
    return out.reshape(B, L, D)


# revision 30
# speedup vs baseline: 1.0263x; 1.0263x over previous
"""MoE layer (16 experts, top-2) + shared SwiGLU MLP on 8 trn2 NeuronCores.

Sharding:
  - MoE experts: expert-parallel — core c owns experts {2c, 2c+1}. The host
    computes the router (0.2% of the FLOPs), gathers each expert's tokens
    (the "all-to-all" happens while building per-core inputs), and the device
    runs both expert FFNs on the gathered tokens.
  - Shared SwiGLU MLP: hybrid 4-way token x 2-way hidden shard. Core c
    handles token quarter (c % 4) and S-half (c // 4); each core emits a
    partial second-matmul output and the host sums the two S-halves.
  - The host applies the top-2 softmax combine weights, scatter-adds expert
    outputs, and adds the shared-expert output.

All matmul operands are bf16 (PSUM accumulates fp32): halves HBM traffic vs
fp32 and enables the PE's automatic fast-weight-load path, and the 2e-2
rel-err budget dwarfs the ~1e-3 bf16 error. Device outputs are bf16 too
(host accumulates in fp32). Every input tensor gets a dedicated SBUF tile
whose DMA is issued up front (sync engine: shared-expert stream; gpsimd:
expert weights + second-layer weights), so the tensor engine never waits on
a recycled buffer. Every operand is laid out host-side exactly as its SBUF
tile (partition-major), so each DMA is one contiguous-row transfer and every
matmul is lhsT.T @ rhs with no on-device transposes.
"""

import os
import numpy as np

import concourse.bacc as bacc
import concourse.mybir as mybir
import concourse.tile as tile
from concourse import bass_utils

AF = mybir.ActivationFunctionType
FP32 = mybir.dt.float32

B, L, D, H, E, S = 2, 2048, 1024, 512, 16, 2048
T = B * L
TOP_K = 2
NCORES = 8
EPC = E // NCORES   # experts per core
PT = 4              # token-shard ways for the shared expert
PS = 2              # hidden(S)-shard ways for the shared expert
TQ = T // PT        # tokens per core for the shared expert (1024)
SH = S // PS        # hidden units per core for the shared expert (1024)

KD = D // 128       # 8 contraction tiles over D
KH = H // 128       # 4 contraction tiles over H
KSH = SH // 128     # 8 s-tiles per core (its S-half)

MM_DTYPE = os.environ.get("KMM_DTYPE", "bf16")
_MM_DT = {
    "fp32": mybir.dt.float32,
    "fp32r": mybir.dt.float32r,
    "bf16": mybir.dt.bfloat16,
}

TRACE = False      # set True (or BASS_TRACE=1) to collect an NTFF profile
LAST = None        # BassKernelResults of the most recent run (for test.py)

_PROG_CACHE = {}


def _chunks(total, step=512):
    """Split ``total`` into near-equal chunks <= step (PSUM bank = 512 fp32)."""
    n = max(1, -(-total // step))
    base = total // n
    rem = total - base * n
    out, off = [], 0
    for i in range(n):
        w = base + (1 if i < rem else 0)
        out.append((off, w))
        off += w
    return out


def _pmajor(a, cols):
    """[K, M] k-major matrix -> [128, (K/128)*M] partition-major image whose
    columns are the K-tiles side by side; ``cols`` = M per tile."""
    K, M = a.shape
    assert M == cols
    return np.ascontiguousarray(
        a.reshape(K // 128, 128, M).transpose(1, 0, 2).reshape(128, -1)
    )


def build_program(C0, C1, mmdt_key=None):
    mmdt = _MM_DT[mmdt_key or MM_DTYPE]
    outdt = mmdt if mmdt == mybir.dt.bfloat16 else FP32
    nc = bacc.Bacc(
        "TRN2", target_bir_lowering=False, debug=False, enable_asserts=False
    )

    CS = (C0, C1)
    xgw = KD * (C0 + C1)

    # [128, (tc*KD + j)*512 + t]: token chunk tc, k-tile j, partition-major
    xq = nc.dram_tensor("xq", [128, 2 * KD * 512], mmdt, kind="ExternalInput").ap()
    # per s-tile: 8 sfc1 k-tiles then 8 sfc2 k-tiles, side by side
    sfc12 = nc.dram_tensor("sfc12", [KSH, 128, 2 * KD * 128], mmdt, kind="ExternalInput").ap()
    # [128, dt*KSH*128 + s]: the core's 8 sfc3 s-tiles per d-tile
    sfc3h = nc.dram_tensor("sfc3h", [128, KD * KSH * 128], mmdt, kind="ExternalInput").ap()
    # slot-0 (wide) expert block [128, KD*C0], then slot-1 block [128, KD*C1]
    xg = nc.dram_tensor("xg", [128, xgw], mmdt, kind="ExternalInput").ap()
    w1b = nc.dram_tensor("w1b", [EPC, 128, KH * KD * 128], mmdt, kind="ExternalInput").ap()
    w2b = nc.dram_tensor("w2b", [EPC, 128, KD * KH * 128], mmdt, kind="ExternalInput").ap()
    pshout = nc.dram_tensor("pshout", [KD, 128, TQ], outdt, kind="ExternalOutput").ap()
    yout = nc.dram_tensor("yout", [128, xgw], outdt, kind="ExternalOutput").ap()

    tch = _chunks(TQ)   # token chunks for the shared expert (2 x 512)
    cchs = (_chunks(C0), _chunks(C1))   # token chunks for the owned experts

    with tile.TileContext(nc) as tc:
        with (
            tc.tile_pool(name="inp", bufs=1) as inp,
            tc.tile_pool(name="gp", bufs=1) as gp,
            tc.tile_pool(name="hp", bufs=1) as hp,
            tc.tile_pool(name="sap", bufs=3) as sap,
            tc.tile_pool(name="obp", bufs=1) as obp,
            tc.tile_pool(name="pop", bufs=3) as pop,
            tc.tile_pool(name="ps", bufs=8, space="PSUM") as ps,
        ):
            # ---- all input DMAs up front into dedicated tiles, one ordered
            # stream on the sync engine (issue order = rough arrival order,
            # which matches first-use order; a second engine's stream would
            # race this one for HBM bandwidth and starve stage A) ----
            s12t = [None] * KSH
            for st in range(KSH):
                s12t[st] = inp.tile([128, 2 * KD * 128], mmdt, tag=f"s12_{st}", name=f"s12_{st}")

            xq_t = inp.tile([128, 2 * KD * 512], mmdt, tag="xq", name="xq_t")

            def xq_sl(j, off, w):
                tc_i, local = off // 512, off % 512
                base = (tc_i * KD + j) * 512 + local
                return xq_t[:, base:base + w]

            xe_t, w1t, w2t = [None, None], [None, None], [None, None]
            for e in range(EPC):
                xe_t[e] = inp.tile([128, KD * CS[e]], mmdt, tag=f"xe{e}", name=f"xe{e}")
                w1t[e] = inp.tile([128, KH * KD * 128], mmdt, tag=f"w1_{e}", name=f"w1_{e}")
                w2t[e] = inp.tile([128, KD * KH * 128], mmdt, tag=f"w2_{e}", name=f"w2_{e}")
            w3t = inp.tile([128, KD * KSH * 128], mmdt, tag="w3", name="w3t")

            # single ordered stream on sync (a second engine's stream races
            # this one for HBM bandwidth). Order = just-in-time arrival for
            # a warm PE that starts at the first group's data gate (~13.5us).
            # Note: do NOT split these transfers finer — partially-written
            # tiles make the scheduler add per-matmul subtile waits that
            # cost ~40ns on every downstream matmul (+25us total).
            qcol = KD * 512
            nc.sync.dma_start(out=s12t[0][:], in_=sfc12[0])
            nc.sync.dma_start(out=xq_t[:, :qcol], in_=xq[:, :qcol])
            nc.sync.dma_start(out=xq_t[:, qcol:], in_=xq[:, qcol:])
            for st in range(1, KSH):
                nc.sync.dma_start(out=s12t[st][:], in_=sfc12[st])
            nc.sync.dma_start(out=xe_t[0][:], in_=xg[:, : KD * C0])
            nc.sync.dma_start(out=w1t[0][:], in_=w1b[0])
            nc.sync.dma_start(out=w2t[0][:], in_=w2b[0])
            nc.sync.dma_start(out=w3t[:], in_=sfc3h[:])
            nc.sync.dma_start(out=xe_t[1][:], in_=xg[:, KD * C0:])
            nc.sync.dma_start(out=w1t[1][:], in_=w1b[1])
            nc.sync.dma_start(out=w2t[1][:], in_=w2b[1])

            # warm-up: ~6us of dependency-free matmuls (on gpsimd-memset
            # tiles) bridge the PE through the first DMA wait so the HAM
            # clock gate opens (1.2 -> 2.4 GHz) right as real work arrives;
            # sized to end at the first group's data gate, because a >2us
            # PE idle gap here would re-throttle the clock.
            dmw = inp.tile([128, 128], mmdt, tag="dmw", name="dmw")
            dmx = inp.tile([128, 512], mmdt, tag="dmx", name="dmx")
            nc.gpsimd.memset(dmw[:], 0.0)
            nc.gpsimd.memset(dmx[:], 0.0)
            # 15 iterations measured best: 19 overran the data gate and
            # delayed the real stream (136.5us vs 132.9us).
            for _ in range(15):
                pd = ps.tile([128, 512], FP32, tag="ps")
                nc.tensor.matmul(pd[:], dmw[:], dmx[:], start=True, stop=True)

            # ---- stage A: g[s, t] = silu(x@sfc1.T) * (x@sfc2.T) ----
            g_t = gp.tile([128, KSH * TQ], mmdt, tag="g", name="g_t")
            for st in range(KSH):
                for off, w in tch:
                    pa = ps.tile([128, 512], FP32, tag="ps")
                    for j in range(KD):
                        nc.tensor.matmul(
                            pa[:, :w], s12t[st][:, j * 128:(j + 1) * 128],
                            xq_sl(j, off, w),
                            start=(j == 0), stop=(j == KD - 1),
                        )
                    sa = sap.tile([128, 512], FP32, tag="sa")
                    nc.scalar.activation(sa[:, :w], pa[:, :w], AF.Silu)
                    pb = ps.tile([128, 512], FP32, tag="ps")
                    for j in range(KD):
                        nc.tensor.matmul(
                            pb[:, :w], s12t[st][:, (KD + j) * 128:(KD + j + 1) * 128],
                            xq_sl(j, off, w),
                            start=(j == 0), stop=(j == KD - 1),
                        )
                    nc.vector.tensor_mul(
                        g_t[:, st * TQ + off:st * TQ + off + w],
                        sa[:, :w], pb[:, :w],
                    )

            # ---- owned experts: y_e = silu(x_e @ w1.T) @ w2.T ----
            def emit_expert(e):
                C = CS[e]
                ybase = 0 if e == 0 else KD * C0
                h_t = hp.tile([128, KH * C], mmdt, tag=f"h{e}", name=f"h{e}")
                for ht in range(KH):
                    for off, w in cchs[e]:
                        ph = ps.tile([128, 512], FP32, tag="ps")
                        for j in range(KD):
                            nc.tensor.matmul(
                                ph[:, :w],
                                w1t[e][:, (ht * KD + j) * 128:(ht * KD + j + 1) * 128],
                                xe_t[e][:, j * C + off:j * C + off + w],
                                start=(j == 0), stop=(j == KD - 1),
                            )
                        nc.scalar.activation(
                            h_t[:, ht * C + off:ht * C + off + w], ph[:, :w], AF.Silu
                        )
                yo = obp.tile([128, KD * C], outdt, tag=f"yo{e}", name=f"yo{e}")
                for dt in range(KD):
                    for off, w in cchs[e]:
                        py = ps.tile([128, 512], FP32, tag="ps")
                        for j in range(KH):
                            nc.tensor.matmul(
                                py[:, :w],
                                w2t[e][:, (dt * KH + j) * 128:(dt * KH + j + 1) * 128],
                                h_t[:, j * C + off:j * C + off + w],
                                start=(j == 0), stop=(j == KH - 1),
                            )
                        nc.vector.tensor_copy(
                            yo[:, dt * C + off:dt * C + off + w], py[:, :w]
                        )
                        if e == 1 and dt == KD - 1:
                            # very last output: per-chunk DMAs so the exit
                            # barrier waits on a half-size final transfer
                            nc.scalar.dma_start(
                                out=yout[:, ybase + dt * C + off:ybase + dt * C + off + w],
                                in_=yo[:, dt * C + off:dt * C + off + w],
                            )
                    if not (e == 1 and dt == KD - 1):
                        nc.scalar.dma_start(
                            out=yout[:, ybase + dt * C:ybase + (dt + 1) * C],
                            in_=yo[:, dt * C:(dt + 1) * C],
                        )

            def emit_stage_b():
                # partial shared second matmul over this core's S-half:
                # pshout[d, t] = sum_{s in half} sfc3[d, s] * g[s, t]
                for dt in range(KD):
                    po = pop.tile([128, TQ], outdt, tag="po")
                    for off, w in tch:
                        pc = ps.tile([128, 512], FP32, tag="ps")
                        for sj in range(KSH):
                            nc.tensor.matmul(
                                pc[:, :w],
                                w3t[:, (dt * KSH + sj) * 128:(dt * KSH + sj + 1) * 128],
                                g_t[:, sj * TQ + off:sj * TQ + off + w],
                                start=(sj == 0), stop=(sj == KSH - 1),
                            )
                        nc.vector.tensor_copy(po[:, off:off + w], pc[:, :w])
                    nc.scalar.dma_start(out=pshout[dt], in_=po[:])

            emit_expert(0)
            emit_stage_b()
            emit_expert(1)

    nc.compile()
    return nc


def kernel(**inputs):
    global LAST
    x = np.ascontiguousarray(np.asarray(inputs["x"], dtype=np.float32))
    gate_w = np.asarray(inputs["gate_w"], dtype=np.float32)
    w1 = np.asarray(inputs["w1"], dtype=np.float32)
    w2 = np.asarray(inputs["w2"], dtype=np.float32)
    sfc1 = np.asarray(inputs["sfc1"], dtype=np.float32)
    sfc2 = np.asarray(inputs["sfc2"], dtype=np.float32)
    sfc3 = np.asarray(inputs["sfc3"], dtype=np.float32)

    xf = x.reshape(T, D)

    # router on host (tiny): top-2 of 16 logits, softmax over the pair
    logits = xf @ gate_w.T
    idx = np.argpartition(-logits, TOP_K, axis=1)[:, :TOP_K]
    lg = np.take_along_axis(logits, idx, axis=1)
    m = lg.max(axis=1, keepdims=True)
    p = np.exp(lg - m)
    wk = (p / p.sum(axis=1, keepdims=True)).astype(np.float32)

    toks, wts = [], []
    for e in range(E):
        sel = idx == e
        rows = np.nonzero(sel.any(axis=1))[0]
        toks.append(rows)
        wts.append(wk[sel])

    # slot assignment: the 8 most-loaded experts take slot 0 (width C0),
    # the 8 least-loaded take slot 1 (width C1 <= C0); core c gets
    # (order[c], order[E-1-c]). Shrinks the padded token count per core
    # from 2*max_load to max_load + median_load.
    order = sorted(range(E), key=lambda e: -len(toks[e]))
    assign = [(order[c], order[E - 1 - c]) for c in range(NCORES)]
    pad = lambda n: max(2 * ((n + 1) // 2), 256)
    C0 = pad(max(len(toks[e]) for e in order[:NCORES]))
    C1 = pad(max(len(toks[e]) for e in order[NCORES:]))
    CS = (C0, C1)

    key = (C0, C1, MM_DTYPE)
    if key not in _PROG_CACHE:
        _PROG_CACHE[key] = build_program(C0, C1, MM_DTYPE)
    nc = _PROG_CACHE[key]
    np_mm = mybir.dt.np(_MM_DT[MM_DTYPE])

    sfc1T = np.ascontiguousarray(sfc1.T)   # [D, S]
    sfc2T = np.ascontiguousarray(sfc2.T)
    sfc3T = np.ascontiguousarray(sfc3.T)   # [S, D]

    # sfc12 per S-half: [KSH, 128, 2*KD*128]
    sfc12_h, sfc3_h = [], []
    for sh in range(PS):
        blk = np.empty((KSH, 128, 2 * KD * 128), np.float32)
        for st in range(KSH):
            s0 = (sh * KSH + st) * 128
            a = sfc1T[:, s0:s0 + 128]    # [D, 128]
            b = sfc2T[:, s0:s0 + 128]
            blk[st, :, : KD * 128] = _pmajor(a, 128)
            blk[st, :, KD * 128:] = _pmajor(b, 128)
        sfc12_h.append(blk.astype(np_mm))
        blk3 = np.empty((KD, 128, KSH * 128), np.float32)
        s0 = sh * SH
        for dt in range(KD):
            # [SH, 128] slice of sfc3T -> partition-major over its s-tiles
            blk3[dt] = _pmajor(
                np.ascontiguousarray(sfc3T[s0:s0 + SH, dt * 128:(dt + 1) * 128]), 128
            )
        # [KD, 128, KSH*128] -> [128, KD*KSH*128] (dt-blocks side by side)
        sfc3_h.append(
            np.ascontiguousarray(
                blk3.transpose(1, 0, 2).reshape(128, KD * KSH * 128)
            ).astype(np_mm)
        )

    in_maps = []
    for c in range(NCORES):
        q, sh = c % PT, c // PT
        xqm = _pmajor(
            np.ascontiguousarray(xf[q * TQ:(q + 1) * TQ].T), TQ
        ).astype(np_mm)
        # [128, KD*TQ] -> [128, (tc*KD + j)*512 + t] (tc-major chunk blocks)
        xqm = np.ascontiguousarray(
            xqm.reshape(128, KD, 2, 512).transpose(0, 2, 1, 3).reshape(128, 2 * KD * 512)
        )
        xg_c = np.zeros((128, KD * (C0 + C1)), np.float32)
        w1_c, w2_c = [], []
        for k in range(EPC):
            e = assign[c][k]
            C = CS[k]
            base = 0 if k == 0 else KD * C0
            rows = toks[e]
            xe = np.zeros((C, D), np.float32)
            xe[: len(rows)] = xf[rows]
            xg_c[:, base:base + KD * C] = _pmajor(np.ascontiguousarray(xe.T), C)
            # w1 tiles keyed (ht, j): col block (ht*KD + j) is k-tile j of
            # w1[e].T's h-tile ht
            w1T = np.ascontiguousarray(w1[e].T)   # [D, H]
            w1m = np.empty((128, KH * KD * 128), np.float32)
            for ht in range(KH):
                w1m[:, ht * KD * 128:(ht + 1) * KD * 128] = _pmajor(
                    np.ascontiguousarray(w1T[:, ht * 128:(ht + 1) * 128]), 128
                )
            w1_c.append(w1m)
            # w2 tiles keyed (dt, hj)
            w2T = np.ascontiguousarray(w2[e].T)   # [H, D]
            w2m = np.empty((128, KD * KH * 128), np.float32)
            for dt in range(KD):
                w2m[:, dt * KH * 128:(dt + 1) * KH * 128] = _pmajor(
                    np.ascontiguousarray(w2T[:, dt * 128:(dt + 1) * 128]), 128
                )
            w2_c.append(w2m)
        in_maps.append(
            {
                "xq": xqm,
                "sfc12": sfc12_h[sh],
                "sfc3h": sfc3_h[sh],
                "xg": xg_c.astype(np_mm),
                "w1b": np.stack(w1_c).astype(np_mm),
                "w2b": np.stack(w2_c).astype(np_mm),
            }
        )

    trace = TRACE or os.environ.get("BASS_TRACE") == "1"
    res = bass_utils.run_bass_kernel_spmd(
        nc, in_maps, core_ids=list(range(NCORES)), trace=trace
    )
    LAST = res
    results = res.results

    out = np.empty((T, D), np.float32)
    for q in range(PT):
        acc = np.asarray(results[q]["pshout"], np.float32).reshape(D, TQ)
        acc = acc + np.asarray(results[PT + q]["pshout"], np.float32).reshape(D, TQ)
        out[q * TQ:(q + 1) * TQ] = acc.T
    for c in range(NCORES):
        yc = np.asarray(results[c]["yout"], np.float32)
        for k in range(EPC):
            e = assign[c][k]
            C = CS[k]
            base = 0 if k == 0 else KD * C0
            load = len(toks[e])
            # [128, KD*C] partition-major -> [D, C]
            yT = yc[:, base:base + KD * C].reshape(128, KD, C).transpose(1, 0, 2).reshape(D, C)
            out[toks[e]] += wts[e][:, None] * yT[:, :load].T
    return out.reshape(B, L, D)
